# revision 1
# baseline (speedup 1.0000x reference)
"""Trainium2 Bass kernel for nn_ARM_28217935134778 (dense_cnn).

Computation (see reference): for each of the 65536 pixels of a 256x256 image,
gather a 7x7 window over 30 channels from two tensors (x: first 24 taps per
channel, x_ups: flat-tail 750 taps), feed the 1470-dim feature through a
1470 -> 2048 -> 6 MLP (ReLU in the middle), then map the 6 outputs to
(mu, scale) pairs.

Implementation: implicit-GEMM convolution, data-parallel over 8 NeuronCores
(each core takes a 32-row horizontal strip of the image, with a 3-row halo
baked into its padded input strip on the host).

Device layout per core:
  - rhs "feature" tiles [128 feats, 512 pixels] built by shifted DMAs from the
    halo-padded strip (one DMA covers several taps x channels).
  - W1 is host-reordered so its rows match the feature order; b1 is folded
    in via an extra constant-1.0 feature row whose W1 row equals b1.
  - Layer 1: out1[hid, pix] accumulated over 12 K-groups per 128-hid block
    (float32r matmuls, full PE rate). ReLU evict PSUM->SBUF on ACT.
  - Layer 2: out2[6, pix] accumulated over the 16 hidden chunks.
  - mu/scale transform on ACT/DVE, output stored feature-major (6, 8192);
    the host transposes/interleaves while gathering.
"""

import numpy as np

import concourse.bass as bass
import concourse.mybir as mybir
import concourse.tile as tile
from concourse import bacc
from concourse.bass_utils import run_bass_kernel_spmd

F32 = mybir.dt.float32
F32R = mybir.dt.float32r

C = 30            # channels
H = W = 256
KW = 7            # window
PAD = 3
CENTER = 24       # causal taps per channel
HID = 2048
NCORES = 8
ROWS_PER_CORE = H // NCORES          # 32
PIX_PER_CORE = ROWS_PER_CORE * W     # 8192
PW = 264                             # padded row width (3 left, 5 right)
PH = ROWS_PER_CORE + 2 * PAD         # 38 padded rows per strip
NPB = PIX_PER_CORE // 512            # 16 pixel blocks (2 image rows each)
NKG = 12                             # K groups (11 x 128 + 1 x 63)
KG_LAST = 63                         # 62 feature rows + 1 bias row
NM = HID // 128                      # 16 hidden blocks


def _build_runs():
    """Feature rows in our contraction order: (tensor_id, di, dj, c0, nch)."""
    runs = []
    for t in range(CENTER):                       # x: taps 0..23, all 30 ch
        runs.append((0, t // KW, t % KW, 0, C))
    for t in range(KW * KW):                      # x_ups tail
        c0 = 15 if t < 34 else 14
        runs.append((1, t // KW, t % KW, c0, C - c0))
    return runs


def _build_perm(runs):
    """Original W1 row index for each position in our feature order."""
    perm = []
    for (tid, di, dj, c0, nch) in runs:
        t = di * KW + dj
        for c in range(c0, c0 + nch):
            perm.append(c * CENTER + t if tid == 0 else c * KW * KW + t)
    assert len(perm) == 1470
    assert sorted(perm) == list(range(1470))
    return perm


def _build_pieces(runs):
    """Split runs at 128-row group boundaries, then merge consecutive taps
    (same di, channel range) into single multi-tap DMA pieces.

    Position 1408 (partition 0 of K-group 11) is reserved for the constant-1
    bias feature row, so feature positions >= 1408 shift up by one."""
    subs = []
    pos = 0
    for (tid, di, dj, c0, nch) in runs:
        left, cs = nch, c0
        while left:
            g, p = divmod(pos if pos < 1408 else pos + 1, 128)
            take = min(left, 128 - p)
            subs.append(dict(g=g, p=p, tid=tid, di=di, dj=dj, c0=cs, nch=take))
            pos += take
            cs += take
            left -= take
    assert pos == 1470
    pieces = []
    for s in subs:
        m = pieces[-1] if pieces else None
        if (m is not None and m["g"] == s["g"] and m["tid"] == s["tid"]
                and m["di"] == s["di"] and m["c0"] == s["c0"]
                and m["nch"] == s["nch"] and s["dj"] == m["dj"] + m["ntap"]
                and s["p"] == m["p"] + m["ntap"] * m["nch"]):
            m["ntap"] += 1
        else:
            pieces.append(dict(**s, ntap=1))
    return pieces


_RUNS = _build_runs()
_PERM = _build_perm(_RUNS)
_PIECES = _build_pieces(_RUNS)


def _build_nc(fbufs=2, hbufs=4, ps1bufs=7, ps2bufs=1, npb=NPB):
    nc = bacc.Bacc("TRN2", target_bir_lowering=False, debug=False)
    xs = nc.dram_tensor("xs", (C, PH, PW), F32R, kind="ExternalInput")
    us = nc.dram_tensor("us", (C, PH, PW), F32R, kind="ExternalInput")
    w1 = nc.dram_tensor("w1", (NKG * 128, HID), F32R, kind="ExternalInput")
    # layer-2 weights, columns [mu0 mu1 mu2 s0 s1 s2]
    w2 = nc.dram_tensor("w2", (HID, 6), F32R, kind="ExternalInput")
    # per-partition scale/bias vectors for the output transform
    # cols: sA bA (mu), sC bC (denominator), sD bD (numerator)
    b3 = nc.dram_tensor("b3", (6, 6), F32, kind="ExternalInput")
    ones = nc.dram_tensor("ones", (1, 520), F32R, kind="ExternalInput")
    o = nc.dram_tensor("o", (6, PIX_PER_CORE), F32, kind="ExternalOutput")
    strip = {0: xs, 1: us}

    with tile.TileContext(nc) as tc:
        with (
            tc.tile_pool(name="wpool", bufs=1) as wpool,
            tc.tile_pool(name="cpool", bufs=1) as cpool,
            tc.tile_pool(name="fpool", bufs=fbufs) as fpool,
            tc.tile_pool(name="hpool", bufs=hbufs) as hpool,
            tc.tile_pool(name="spool", bufs=1) as spool,
            tc.tile_pool(name="opool", bufs=2) as opool,
            tc.tile_pool(name="ps1pool", bufs=ps1bufs, space="PSUM") as ps1pool,
            tc.tile_pool(name="ps2pool", bufs=ps2bufs, space="PSUM") as ps2pool,
        ):
            w1_sb = wpool.tile([128, NKG, HID], F32R)
            # m-major lazy load: chunk m (all K-groups, one 128-wide hidden
            # block, 768KB) is issued just before pixel-block 0 consumes it,
            # so the matmul stream pipelines against W1 arrival instead of
            # waiting for the whole 12MB
            w1v = w1.ap().rearrange("(g p) h -> p g h", p=128)

            def load_w1_chunk(m):
                nc.sync.dma_start(w1_sb[:, :, m * 128:(m + 1) * 128],
                                  w1v[:, :, m * 128:(m + 1) * 128])
            # chunk stride padded to 8 floats so each lhsT slice is 32B-aligned
            w2_sb = wpool.tile([128, NM, 8], F32R)
            nc.sync.dma_start(w2_sb[:, :, 0:6],
                              w2.ap().rearrange("(m p) o -> p m o", p=128))
            b3_sb = cpool.tile([6, 6], F32)
            nc.sync.dma_start(b3_sb[:], b3.ap()[:])
            zb = cpool.tile([128, 1], F32)
            nc.any.memset(zb[:], 0.0)

            for pb in range(npb):
                R = 2 * pb  # first image row (strip-local) of this block
                # free layout per K-row: [2 rows, 264 cols] — a single
                # contiguous 520-element DRAM read covers both rows (the 8
                # inter-row pad columns land in [*, 0, 256:264] and are never
                # read by the matmuls).
                feat = fpool.tile([128, NKG, 2, 264], F32R)
                for pc in _PIECES:
                    t = strip[pc["tid"]]
                    off = (pc["c0"] * PH * PW + (R + pc["di"]) * PW + pc["dj"])
                    src = bass.AP(
                        t,
                        off,
                        [[1, pc["ntap"]], [PH * PW, pc["nch"]], [1, 520]],
                    )
                    npart = pc["ntap"] * pc["nch"]
                    # dst: partitions p..p+npart, contiguous 520-elem span
                    # starting at [g, 0, 0]
                    dst = feat[pc["p"]:pc["p"] + npart, pc["g"], :, :]
                    dst = bass.AP(dst.tensor, dst.offset,
                                  [list(dst.ap[0]), [1, 520]])
                    nc.sync.dma_start(dst, src)
                # bias feature row: constant 1.0 (W1 row 1408 = b1)
                brow = feat[0:1, NKG - 1, :, :]
                brow = bass.AP(brow.tensor, brow.offset,
                               [list(brow.ap[0]), [1, 520]])
                nc.scalar.dma_start(brow, ones.ap()[:])

                ps2 = ps2pool.tile([6, 512], F32)
                for m in range(NM):
                    if pb == 0:
                        load_w1_chunk(m)
                    ps = ps1pool.tile([128, 512], F32)
                    for g in range(NKG):
                        kg = 128 if g < NKG - 1 else KG_LAST
                        nc.tensor.matmul(
                            ps[:],
                            w1_sb[0:kg, g, m * 128:(m + 1) * 128],
                            feat[0:kg, g, :, 0:256],
                            start=(g == 0),
                            stop=(g == NKG - 1),
                        )
                    h = hpool.tile([128, 512], F32R)
                    nc.scalar.activation(
                        h[:], ps[:], mybir.ActivationFunctionType.Relu,
                        bias=zb[:],
                    )
                    nc.tensor.matmul(
                        ps2[:],
                        w2_sb[:, m, 0:6],
                        h[:],
                        start=(m == 0),
                        stop=(m == NM - 1),
                        skip_group_check=True,
                    )

                # All transform ops run on partitions 0:6 with per-partition
                # scale/bias vectors; rows that don't apply get neutral values
                # (scale 0, bias 1) so every lane stays finite.
                # mu rows 0:3: (raw + b2_mu)*255 ; d rows 3:6: 1.1-(raw+b2_s)
                outm = spool.tile([6, 512], F32, tag="outm")
                nc.scalar.activation(
                    outm[:], ps2[:],
                    mybir.ActivationFunctionType.Identity,
                    bias=b3_sb[:, 1:2], scale=b3_sb[:, 0:1],
                )
                d = spool.tile([6, 512], F32, tag="d")
                nc.scalar.activation(
                    d[:], ps2[:],
                    mybir.ActivationFunctionType.Identity,
                    bias=b3_sb[:, 3:4], scale=b3_sb[:, 2:3],
                )
                r = spool.tile([6, 512], F32, tag="r")
                nc.vector.reciprocal(r[:], d[:])
                # n rows 3:6: 100*(raw + b2_s)
                n = spool.tile([6, 512], F32, tag="n")
                nc.scalar.activation(
                    n[:], ps2[:],
                    mybir.ActivationFunctionType.Identity,
                    bias=b3_sb[:, 5:6], scale=b3_sb[:, 4:5],
                )
                sc = spool.tile([6, 512], F32, tag="sc")
                nc.vector.tensor_mul(sc[:], n[:], r[:])
                scc = spool.tile([6, 512], F32, tag="scc")
                nc.vector.tensor_scalar(
                    scc[:], sc[:], 1000.0, 1e-8,
                    op0=mybir.AluOpType.min, op1=mybir.AluOpType.max,
                )
                pbs = slice(pb * 512, (pb + 1) * 512)
                nc.scalar.dma_start(o.ap()[0:3, pbs], outm[0:3, :])
                nc.scalar.dma_start(o.ap()[3:6, pbs], scc[3:6, :])

    nc.compile()
    return nc


_NC_CACHE = None


def _get_nc():
    global _NC_CACHE
    if _NC_CACHE is None:
        _NC_CACHE = _build_nc()
    return _NC_CACHE


def _prep_host_inputs(x, x_ups, W1, b1, W2, b2):
    x = np.asarray(x)
    x_ups = np.asarray(x_ups)
    # halo-padded full images
    def pad_full(a):
        p = np.zeros((C, H + 2 * PAD, PW), np.float32)
        p[:, PAD:PAD + H, PAD:PAD + W] = a[0]
        return p

    xp = pad_full(x)
    up = pad_full(x_ups)

    # reordered W1; bias row (=b1) at position 1408; zero-pad to 12*128 rows
    W1g = np.zeros((NKG * 128, HID), np.float32)
    W1p = np.asarray(W1)[_PERM]
    W1g[:1408] = W1p[:1408]
    W1g[1408] = np.asarray(b1)
    W1g[1409:1471] = W1p[1408:]
    # W2 column-reordered: [mu0 mu1 mu2 s0 s1 s2]
    W2a = np.asarray(W2).astype(np.float32)
    b2a = np.asarray(b2).astype(np.float32)
    W2r = np.ascontiguousarray(W2a[:, [0, 2, 4, 1, 3, 5]])
    b3 = np.zeros((6, 6), np.float32)
    b3[0:3, 0] = 255.0                        # sA (mu scale)
    b3[0:3, 1] = b2a[[0, 2, 4]] * 255.0       # bA (mu bias)
    b3[3:6, 2] = -1.0                         # sC (d scale)
    b3[0:3, 3] = 1.0                          # bC neutral rows
    b3[3:6, 3] = 1.1 - b2a[[1, 3, 5]]         # bC (d bias)
    b3[3:6, 4] = 100.0                        # sD (n scale)
    b3[0:3, 5] = 1.0                          # bD neutral rows
    b3[3:6, 5] = 100.0 * b2a[[1, 3, 5]]       # bD (n bias)

    in_maps = []
    for k in range(NCORES):
        r0 = k * ROWS_PER_CORE
        in_maps.append({
            "xs": np.ascontiguousarray(xp[:, r0:r0 + PH, :]),
            "us": np.ascontiguousarray(up[:, r0:r0 + PH, :]),
            "w1": W1g,
            "w2": W2r,
            "b3": b3,
            "ones": np.ones((1, 520), np.float32),
        })
    return in_maps


def kernel(x, x_ups, W1, b1, W2, b2):
    nc = _get_nc()
    in_maps = _prep_host_inputs(x, x_ups, W1, b1, W2, b2)
    res = run_bass_kernel_spmd(nc, in_maps, core_ids=list(range(NCORES)))
    ocs = np.stack([res.results[k]["o"] for k in range(NCORES)])  # (8, 6, 8192)
    flat = ocs.transpose(0, 2, 1).reshape(H * W, 6)               # (65536, 6)
    out = flat.reshape(H * W, 2, 3).transpose(0, 2, 1)            # (65536, 3, 2)
    return np.ascontiguousarray(out[None]).astype(np.float32)     # (1, 65536, 3, 2)



# revision 2
# speedup vs baseline: 2.9067x; 2.9067x over previous
"""Trainium2 Bass kernel for nn_ARM_28217935134778 (dense_cnn).

Computation (see reference): for each of the 65536 pixels of a 256x256 image,
gather a 7x7 window over 30 channels from two tensors (x: first 24 taps per
channel, x_ups: flat-tail 750 taps), feed the 1470-dim feature through a
1470 -> 2048 -> 6 MLP (ReLU in the middle), then map the 6 outputs to
(mu, scale) pairs.

Implementation: implicit-GEMM convolution, data-parallel over 8 NeuronCores
(each core takes a 32-row horizontal strip of the image, with the needed halo
rows baked into its padded input strip on the host).

The dominant cost under the axon tunnel is host->device staging (~37 MB/s), so
inputs are kept minimal:
  - W1 is shipped SHARDED (192 rows per core) and AllGathered on-device over
    NeuronLink, instead of replicating the 12.6 MB reordered matrix 8x.
  - The x strip only carries the 35 rows that causal taps 0..23 can touch;
    the x_ups strip only carries channels 14..29 (the flat tail [:, 720:]
    never reads channels 0..13).

Device layout per core:
  - rhs "feature" tiles [128 feats, 512 pixels] built by shifted DMAs from the
    halo-padded strips (one DMA covers several taps x channels).
  - W1 is host-reordered so its rows match the feature order; b1 is folded
    in via an extra constant-1.0 feature row whose W1 row equals b1.
  - Layer 1: out1[hid, pix] accumulated over 12 K-groups per 128-hid block
    (float32r matmuls, full PE rate). ReLU evict PSUM->SBUF on ACT.
  - Layer 2: out2[6, pix] accumulated over the 16 hidden chunks.
  - mu/scale transform on ACT/DVE, output stored feature-major (6, 8192);
    the host transposes/interleaves while gathering.
"""

import numpy as np

import concourse.bass as bass
import concourse.mybir as mybir
import concourse.tile as tile
from concourse import bacc
from concourse.bass_utils import run_bass_kernel_spmd

F32 = mybir.dt.float32
F32R = mybir.dt.float32r

C = 30            # channels
H = W = 256
KW = 7            # window
PAD = 3
CENTER = 24       # causal taps per channel
HID = 2048
NCORES = 8
ROWS_PER_CORE = H // NCORES          # 32
PIX_PER_CORE = ROWS_PER_CORE * W     # 8192
PW = 264                             # padded row width (3 left, 5 right)
PH_X = ROWS_PER_CORE + PAD           # 35 rows: x taps only reach di 0..3
PH_U = ROWS_PER_CORE + 2 * PAD       # 38 rows for the full-window ups taps
CU0 = 14                             # first x_ups channel actually used
CU = C - CU0                         # 16 shipped x_ups channels
NPB = PIX_PER_CORE // 512            # 16 pixel blocks (2 image rows each)
NKG = 12                             # K groups (11 x 128 + 1 x 63)
KG_LAST = 63                         # 62 feature rows + 1 bias row
NM = HID // 128                      # 16 hidden blocks
W1ROWS = NKG * 128                   # 1536 padded W1 rows
W1SH = W1ROWS // NCORES              # 192-row W1 shard shipped per core


def _build_runs():
    """Feature rows in our contraction order: (tensor_id, di, dj, c0, nch)."""
    runs = []
    for t in range(CENTER):                       # x: taps 0..23, all 30 ch
        runs.append((0, t // KW, t % KW, 0, C))
    for t in range(KW * KW):                      # x_ups tail
        c0 = 15 if t < 34 else 14
        runs.append((1, t // KW, t % KW, c0, C - c0))
    return runs


def _build_perm(runs):
    """Original W1 row index for each position in our feature order."""
    perm = []
    for (tid, di, dj, c0, nch) in runs:
        t = di * KW + dj
        for c in range(c0, c0 + nch):
            perm.append(c * CENTER + t if tid == 0 else c * KW * KW + t)
    assert len(perm) == 1470
    assert sorted(perm) == list(range(1470))
    return perm


def _build_pieces(runs):
    """Split runs at 128-row group boundaries, then merge consecutive taps
    (same di, channel range) into single multi-tap DMA pieces.

    Position 1408 (partition 0 of K-group 11) is reserved for the constant-1
    bias feature row, so feature positions >= 1408 shift up by one."""
    subs = []
    pos = 0
    for (tid, di, dj, c0, nch) in runs:
        left, cs = nch, c0
        while left:
            g, p = divmod(pos if pos < 1408 else pos + 1, 128)
            take = min(left, 128 - p)
            subs.append(dict(g=g, p=p, tid=tid, di=di, dj=dj, c0=cs, nch=take))
            pos += take
            cs += take
            left -= take
    assert pos == 1470
    pieces = []
    for s in subs:
        m = pieces[-1] if pieces else None
        if (m is not None and m["g"] == s["g"] and m["tid"] == s["tid"]
                and m["di"] == s["di"] and m["c0"] == s["c0"]
                and m["nch"] == s["nch"] and s["dj"] == m["dj"] + m["ntap"]
                and s["p"] == m["p"] + m["ntap"] * m["nch"]):
            m["ntap"] += 1
        else:
            pieces.append(dict(**s, ntap=1))
    return pieces


_RUNS = _build_runs()
_PERM = _build_perm(_RUNS)
_PIECES = _build_pieces(_RUNS)


def _build_nc(fbufs=2, hbufs=4, ps1bufs=7, ps2bufs=1, npb=NPB):
    nc = bacc.Bacc("TRN2", target_bir_lowering=False, debug=False,
                   num_devices=NCORES)
    xs = nc.dram_tensor("xs", (C, PH_X, PW), F32R, kind="ExternalInput")
    us = nc.dram_tensor("us", (CU, PH_U, PW), F32R, kind="ExternalInput")
    # this core's 192-row shard of the reordered W1; AllGathered on-device
    w1s = nc.dram_tensor("w1s", (W1SH, HID), F32R, kind="ExternalInput")
    # layer-2 weights, columns [mu0 mu1 mu2 s0 s1 s2]
    w2 = nc.dram_tensor("w2", (HID, 6), F32R, kind="ExternalInput")
    # per-partition scale/bias vectors for the output transform
    # cols: sA bA (mu), sC bC (denominator), sD bD (numerator)
    b3 = nc.dram_tensor("b3", (6, 6), F32, kind="ExternalInput")
    ones = nc.dram_tensor("ones", (1, 520), F32R, kind="ExternalInput")
    o = nc.dram_tensor("o", (6, PIX_PER_CORE), F32, kind="ExternalOutput")
    strip = {0: xs, 1: us}
    sdim = {0: (PH_X, 0), 1: (PH_U, CU0)}

    with tile.TileContext(nc) as tc:
        with (
            tc.tile_pool(name="dpool", bufs=1, space="DRAM") as dpool,
            tc.tile_pool(name="wpool", bufs=1) as wpool,
            tc.tile_pool(name="cpool", bufs=1) as cpool,
            tc.tile_pool(name="fpool", bufs=fbufs) as fpool,
            tc.tile_pool(name="hpool", bufs=hbufs) as hpool,
            tc.tile_pool(name="spool", bufs=1) as spool,
            tc.tile_pool(name="opool", bufs=2) as opool,
            tc.tile_pool(name="ps1pool", bufs=ps1bufs, space="PSUM") as ps1pool,
            tc.tile_pool(name="ps2pool", bufs=ps2bufs, space="PSUM") as ps2pool,
        ):
            # --- W1 AllGather: shard (192, 2048) per core -> full (1536, 2048)
            w1_cc_in = dpool.tile([W1SH, HID], F32R)
            nc.sync.dma_start(w1_cc_in[:], w1s.ap()[:])
            w1_full = dpool.tile([W1ROWS, HID], F32R, addr_space="Shared")
            nc.gpsimd.collective_compute(
                "AllGather",
                mybir.AluOpType.bypass,
                replica_groups=[list(range(NCORES))],
                ins=[w1_cc_in[:].opt()],
                outs=[w1_full[:].opt()],
            )

            w1_sb = wpool.tile([128, NKG, HID], F32R)
            # m-major lazy load: chunk m (all K-groups, one 128-wide hidden
            # block, 768KB) is issued just before pixel-block 0 consumes it,
            # so the matmul stream pipelines against the AllGather instead of
            # waiting for the whole 12MB
            w1v = w1_full.rearrange("(g p) h -> p g h", p=128)

            def load_w1_chunk(m):
                nc.sync.dma_start(w1_sb[:, :, m * 128:(m + 1) * 128],
                                  w1v[:, :, m * 128:(m + 1) * 128])
            # chunk stride padded to 8 floats so each lhsT slice is 32B-aligned
            w2_sb = wpool.tile([128, NM, 8], F32R)
            nc.sync.dma_start(w2_sb[:, :, 0:6],
                              w2.ap().rearrange("(m p) o -> p m o", p=128))
            b3_sb = cpool.tile([6, 6], F32)
            nc.sync.dma_start(b3_sb[:], b3.ap()[:])
            zb = cpool.tile([128, 1], F32)
            nc.any.memset(zb[:], 0.0)

            for pb in range(npb):
                R = 2 * pb  # first image row (strip-local) of this block
                # free layout per K-row: [2 rows, 264 cols] — a single
                # contiguous 520-element DRAM read covers both rows (the 8
                # inter-row pad columns land in [*, 0, 256:264] and are never
                # read by the matmuls).
                feat = fpool.tile([128, NKG, 2, 264], F32R)
                for pc in _PIECES:
                    t = strip[pc["tid"]]
                    ph, cbase = sdim[pc["tid"]]
                    off = ((pc["c0"] - cbase) * ph * PW
                           + (R + pc["di"]) * PW + pc["dj"])
                    src = bass.AP(
                        t,
                        off,
                        [[1, pc["ntap"]], [ph * PW, pc["nch"]], [1, 520]],
                    )
                    npart = pc["ntap"] * pc["nch"]
                    # dst: partitions p..p+npart, contiguous 520-elem span
                    # starting at [g, 0, 0]
                    dst = feat[pc["p"]:pc["p"] + npart, pc["g"], :, :]
                    dst = bass.AP(dst.tensor, dst.offset,
                                  [list(dst.ap[0]), [1, 520]])
                    nc.sync.dma_start(dst, src)
                # bias feature row: constant 1.0 (W1 row 1408 = b1)
                brow = feat[0:1, NKG - 1, :, :]
                brow = bass.AP(brow.tensor, brow.offset,
                               [list(brow.ap[0]), [1, 520]])
                nc.scalar.dma_start(brow, ones.ap()[:])

                ps2 = ps2pool.tile([6, 512], F32)
                for m in range(NM):
                    if pb == 0:
                        load_w1_chunk(m)
                    ps = ps1pool.tile([128, 512], F32)
                    for g in range(NKG):
                        kg = 128 if g < NKG - 1 else KG_LAST
                        nc.tensor.matmul(
                            ps[:],
                            w1_sb[0:kg, g, m * 128:(m + 1) * 128],
                            feat[0:kg, g, :, 0:256],
                            start=(g == 0),
                            stop=(g == NKG - 1),
                        )
                    h = hpool.tile([128, 512], F32R)
                    nc.scalar.activation(
                        h[:], ps[:], mybir.ActivationFunctionType.Relu,
                        bias=zb[:],
                    )
                    nc.tensor.matmul(
                        ps2[:],
                        w2_sb[:, m, 0:6],
                        h[:],
                        start=(m == 0),
                        stop=(m == NM - 1),
                        skip_group_check=True,
                    )

                # All transform ops run on partitions 0:6 with per-partition
                # scale/bias vectors; rows that don't apply get neutral values
                # (scale 0, bias 1) so every lane stays finite.
                # mu rows 0:3: (raw + b2_mu)*255 ; d rows 3:6: 1.1-(raw+b2_s)
                outm = spool.tile([6, 512], F32, tag="outm")
                nc.scalar.activation(
                    outm[:], ps2[:],
                    mybir.ActivationFunctionType.Identity,
                    bias=b3_sb[:, 1:2], scale=b3_sb[:, 0:1],
                )
                d = spool.tile([6, 512], F32, tag="d")
                nc.scalar.activation(
                    d[:], ps2[:],
                    mybir.ActivationFunctionType.Identity,
                    bias=b3_sb[:, 3:4], scale=b3_sb[:, 2:3],
                )
                r = spool.tile([6, 512], F32, tag="r")
                nc.vector.reciprocal(r[:], d[:])
                # n rows 3:6: 100*(raw + b2_s)
                n = spool.tile([6, 512], F32, tag="n")
                nc.scalar.activation(
                    n[:], ps2[:],
                    mybir.ActivationFunctionType.Identity,
                    bias=b3_sb[:, 5:6], scale=b3_sb[:, 4:5],
                )
                sc = spool.tile([6, 512], F32, tag="sc")
                nc.vector.tensor_mul(sc[:], n[:], r[:])
                scc = spool.tile([6, 512], F32, tag="scc")
                nc.vector.tensor_scalar(
                    scc[:], sc[:], 1000.0, 1e-8,
                    op0=mybir.AluOpType.min, op1=mybir.AluOpType.max,
                )
                pbs = slice(pb * 512, (pb + 1) * 512)
                nc.scalar.dma_start(o.ap()[0:3, pbs], outm[0:3, :])
                nc.scalar.dma_start(o.ap()[3:6, pbs], scc[3:6, :])

    nc.compile()
    return nc


_NC_CACHE = None


def _get_nc():
    global _NC_CACHE
    if _NC_CACHE is None:
        _NC_CACHE = _build_nc()
    return _NC_CACHE


def _prep_host_inputs(x, x_ups, W1, b1, W2, b2):
    x = np.asarray(x)
    x_ups = np.asarray(x_ups)
    # halo-padded full images
    def pad_full(a, c0=0):
        nch = C - c0
        p = np.zeros((nch, H + 2 * PAD, PW), np.float32)
        p[:, PAD:PAD + H, PAD:PAD + W] = a[0, c0:]
        return p

    xp = pad_full(x)
    up = pad_full(x_ups, CU0)

    # reordered W1; bias row (=b1) at position 1408; zero-pad to 12*128 rows
    W1g = np.zeros((W1ROWS, HID), np.float32)
    W1p = np.asarray(W1)[_PERM]
    W1g[:1408] = W1p[:1408]
    W1g[1408] = np.asarray(b1)
    W1g[1409:1471] = W1p[1408:]
    # W2 column-reordered: [mu0 mu1 mu2 s0 s1 s2]
    W2a = np.asarray(W2).astype(np.float32)
    b2a = np.asarray(b2).astype(np.float32)
    W2r = np.ascontiguousarray(W2a[:, [0, 2, 4, 1, 3, 5]])
    b3 = np.zeros((6, 6), np.float32)
    b3[0:3, 0] = 255.0                        # sA (mu scale)
    b3[0:3, 1] = b2a[[0, 2, 4]] * 255.0       # bA (mu bias)
    b3[3:6, 2] = -1.0                         # sC (d scale)
    b3[0:3, 3] = 1.0                          # bC neutral rows
    b3[3:6, 3] = 1.1 - b2a[[1, 3, 5]]         # bC (d bias)
    b3[3:6, 4] = 100.0                        # sD (n scale)
    b3[0:3, 5] = 1.0                          # bD neutral rows
    b3[3:6, 5] = 100.0 * b2a[[1, 3, 5]]       # bD (n bias)

    in_maps = []
    for k in range(NCORES):
        r0 = k * ROWS_PER_CORE
        in_maps.append({
            "xs": np.ascontiguousarray(xp[:, r0:r0 + PH_X, :]),
            "us": np.ascontiguousarray(up[:, r0:r0 + PH_U, :]),
            "w1s": np.ascontiguousarray(W1g[k * W1SH:(k + 1) * W1SH]),
            "w2": W2r,
            "b3": b3,
            "ones": np.ones((1, 520), np.float32),
        })
    return in_maps


def kernel(x, x_ups, W1, b1, W2, b2):
    nc = _get_nc()
    in_maps = _prep_host_inputs(x, x_ups, W1, b1, W2, b2)
    res = run_bass_kernel_spmd(nc, in_maps, core_ids=list(range(NCORES)))
    ocs = np.stack([res.results[k]["o"] for k in range(NCORES)])  # (8, 6, 8192)
    flat = ocs.transpose(0, 2, 1).reshape(H * W, 6)               # (65536, 6)
    out = flat.reshape(H * W, 2, 3).transpose(0, 2, 1)            # (65536, 3, 2)
    return np.ascontiguousarray(out[None]).astype(np.float32)     # (1, 65536, 3, 2)


# revision 7
# speedup vs baseline: 4.1947x; 1.4431x over previous
"""Trainium2 Bass kernel for nn_ARM_28217935134778 (dense_cnn).

Computation (see reference): for each of the 65536 pixels of a 256x256 image,
gather a 7x7 window over 30 channels from two tensors (x: first 24 taps per
channel, x_ups: flat-tail 750 taps), feed the 1470-dim feature through a
1470 -> 2048 -> 6 MLP (ReLU in the middle), then map the 6 outputs to
(mu, scale) pairs.

Implementation: implicit-GEMM convolution, data-parallel over 8 NeuronCores
(each core takes a 32-row horizontal strip of the image, with the needed halo
rows baked into its padded input strip on the host).

The dominant cost under the axon tunnel is host->device staging (~37 MB/s), so
inputs are kept minimal:
  - W1 is shipped SHARDED (192 rows per core) and AllGathered on-device over
    NeuronLink, instead of replicating the 12.6 MB reordered matrix 8x.
  - The x strip only carries the 35 rows that causal taps 0..23 can touch;
    the x_ups strip only carries channels 14..29 (the flat tail [:, 720:]
    never reads channels 0..13).

Device layout per core:
  - rhs "feature" tiles [128 feats, 512 pixels] built by shifted DMAs from the
    halo-padded strips (one DMA covers several taps x channels).
  - W1 is host-reordered so its rows match the feature order; b1 is folded
    in via an extra constant-1.0 feature row whose W1 row equals b1.
  - Layer 1: out1[hid, pix] accumulated over 12 K-groups per 128-hid block
    (float32r matmuls, full PE rate). ReLU evict PSUM->SBUF on ACT.
  - Layer 2: out2[6, pix] accumulated over the 16 hidden chunks.
  - mu/scale transform on ACT/DVE, output stored feature-major (6, 8192);
    the host transposes/interleaves while gathering.
"""

import numpy as np

try:
    # The repeat-call cost under axon is dominated by staging + the fresh
    # jax.jit that run_bass_kernel_spmd builds per call; the persistent
    # compilation cache turns the per-call XLA re-compile into a disk hit.
    import tempfile as _tempfile
    import jax as _jax
    _jax.config.update("jax_compilation_cache_dir",
                       _tempfile.gettempdir() + "/jax_comp_cache")
    _jax.config.update("jax_persistent_cache_min_entry_size_bytes", -1)
    _jax.config.update("jax_persistent_cache_min_compile_time_secs", 0.0)
except Exception:
    pass

import concourse.bass as bass
import concourse.mybir as mybir
import concourse.tile as tile
from concourse import bacc
from concourse.bass_utils import run_bass_kernel_spmd

F32 = mybir.dt.float32
F32R = mybir.dt.float32r
F16 = mybir.dt.float16

C = 30            # channels
H = W = 256
KW = 7            # window
PAD = 3
CENTER = 24       # causal taps per channel
HID = 2048
NCORES = 8
ROWS_PER_CORE = H // NCORES          # 32
PIX_PER_CORE = ROWS_PER_CORE * W     # 8192
PW = 264                             # padded row width (3 left, 5 right)
PH_X = ROWS_PER_CORE + PAD           # 35 rows: x taps only reach di 0..3
PH_U = ROWS_PER_CORE + 2 * PAD       # 38 rows for the full-window ups taps
CU0 = 14                             # first x_ups channel actually used
CU = C - CU0                         # 16 shipped x_ups channels
NPB = PIX_PER_CORE // 512            # 16 pixel blocks (2 image rows each)
NKG = 12                             # K groups (11 x 128 + 1 x 63)
KG_LAST = 63                         # 62 feature rows + 1 bias row
NM = HID // 128                      # 16 hidden blocks
W1ROWS = NKG * 128                   # 1536 padded W1 rows
W1SH = W1ROWS // NCORES              # 192-row W1 shard shipped per core


def _build_runs():
    """Feature rows in our contraction order: (tensor_id, di, dj, c0, nch)."""
    runs = []
    for t in range(CENTER):                       # x: taps 0..23, all 30 ch
        runs.append((0, t // KW, t % KW, 0, C))
    for t in range(KW * KW):                      # x_ups tail
        c0 = 15 if t < 34 else 14
        runs.append((1, t // KW, t % KW, c0, C - c0))
    return runs


def _build_perm(runs):
    """Original W1 row index for each position in our feature order."""
    perm = []
    for (tid, di, dj, c0, nch) in runs:
        t = di * KW + dj
        for c in range(c0, c0 + nch):
            perm.append(c * CENTER + t if tid == 0 else c * KW * KW + t)
    assert len(perm) == 1470
    assert sorted(perm) == list(range(1470))
    return perm


def _build_pieces(runs):
    """Split runs at 128-row group boundaries, then merge consecutive taps
    (same di, channel range) into single multi-tap DMA pieces.

    Position 1408 (partition 0 of K-group 11) is reserved for the constant-1
    bias feature row, so feature positions >= 1408 shift up by one."""
    subs = []
    pos = 0
    for (tid, di, dj, c0, nch) in runs:
        left, cs = nch, c0
        while left:
            g, p = divmod(pos if pos < 1408 else pos + 1, 128)
            take = min(left, 128 - p)
            subs.append(dict(g=g, p=p, tid=tid, di=di, dj=dj, c0=cs, nch=take))
            pos += take
            cs += take
            left -= take
    assert pos == 1470
    pieces = []
    for s in subs:
        m = pieces[-1] if pieces else None
        if (m is not None and m["g"] == s["g"] and m["tid"] == s["tid"]
                and m["di"] == s["di"] and m["c0"] == s["c0"]
                and m["nch"] == s["nch"] and s["dj"] == m["dj"] + m["ntap"]
                and s["p"] == m["p"] + m["ntap"] * m["nch"]):
            m["ntap"] += 1
        else:
            pieces.append(dict(**s, ntap=1))
    return pieces


_RUNS = _build_runs()
_PERM = _build_perm(_RUNS)
_PIECES = _build_pieces(_RUNS)


def _build_nc(fbufs=2, hbufs=4, ps1bufs=7, ps2bufs=1, npb=NPB):
    nc = bacc.Bacc("TRN2", target_bir_lowering=False, debug=False,
                   num_devices=NCORES)
    xs = nc.dram_tensor("xs", (C, PH_X, PW), F32R, kind="ExternalInput")
    us = nc.dram_tensor("us", (CU, PH_U, PW), F32R, kind="ExternalInput")
    # this core's 192-row shard of the reordered W1; AllGathered on-device
    w1s = nc.dram_tensor("w1s", (W1SH, HID), F32R, kind="ExternalInput")
    # layer-2 weights, columns [mu0 mu1 mu2 s0 s1 s2]
    w2 = nc.dram_tensor("w2", (HID, 6), F32R, kind="ExternalInput")
    # per-partition scale/bias vectors for the output transform
    # cols: sA bA (mu), sC bC (denominator), sD bD (numerator)
    b3 = nc.dram_tensor("b3", (6, 6), F32, kind="ExternalInput")
    ones = nc.dram_tensor("ones", (1, 520), F32R, kind="ExternalInput")
    # fp16 output: halves the donated-zero upload and the result fetch through
    # the tunnel; adds at most 0.25 absolute rounding on values <= 1000.
    o = nc.dram_tensor("o", (6, PIX_PER_CORE), F16, kind="ExternalOutput")
    strip = {0: xs, 1: us}
    sdim = {0: (PH_X, 0), 1: (PH_U, CU0)}

    with tile.TileContext(nc) as tc:
        with (
            tc.tile_pool(name="dpool", bufs=1, space="DRAM") as dpool,
            tc.tile_pool(name="wpool", bufs=1) as wpool,
            tc.tile_pool(name="cpool", bufs=1) as cpool,
            tc.tile_pool(name="fpool", bufs=fbufs) as fpool,
            tc.tile_pool(name="hpool", bufs=hbufs) as hpool,
            tc.tile_pool(name="spool", bufs=1) as spool,
            tc.tile_pool(name="opool", bufs=2) as opool,
            tc.tile_pool(name="ps1pool", bufs=ps1bufs, space="PSUM") as ps1pool,
            tc.tile_pool(name="ps2pool", bufs=ps2bufs, space="PSUM") as ps2pool,
        ):
            # --- W1 AllGather: shard (192, 2048) per core -> full (1536, 2048)
            w1_cc_in = dpool.tile([W1SH, HID], F32R)
            nc.sync.dma_start(w1_cc_in[:], w1s.ap()[:])
            w1_full = dpool.tile([W1ROWS, HID], F32R, addr_space="Shared")
            nc.gpsimd.collective_compute(
                "AllGather",
                mybir.AluOpType.bypass,
                replica_groups=[list(range(NCORES))],
                ins=[w1_cc_in[:].opt()],
                outs=[w1_full[:].opt()],
            )

            w1_sb = wpool.tile([128, NKG, HID], F32R)
            # m-major lazy load: chunk m (all K-groups, one 128-wide hidden
            # block, 768KB) is issued just before pixel-block 0 consumes it,
            # so the matmul stream pipelines against the AllGather instead of
            # waiting for the whole 12MB
            w1v = w1_full.rearrange("(g p) h -> p g h", p=128)

            def load_w1_chunk(m):
                nc.sync.dma_start(w1_sb[:, :, m * 128:(m + 1) * 128],
                                  w1v[:, :, m * 128:(m + 1) * 128])
            # chunk stride padded to 8 floats so each lhsT slice is 32B-aligned
            w2_sb = wpool.tile([128, NM, 8], F32R)
            nc.sync.dma_start(w2_sb[:, :, 0:6],
                              w2.ap().rearrange("(m p) o -> p m o", p=128))
            b3_sb = cpool.tile([6, 6], F32)
            nc.sync.dma_start(b3_sb[:], b3.ap()[:])
            zb = cpool.tile([128, 1], F32)
            nc.any.memset(zb[:], 0.0)

            for pb in range(npb):
                R = 2 * pb  # first image row (strip-local) of this block
                # free layout per K-row: [2 rows, 264 cols] — a single
                # contiguous 520-element DRAM read covers both rows (the 8
                # inter-row pad columns land in [*, 0, 256:264] and are never
                # read by the matmuls).
                feat = fpool.tile([128, NKG, 2, 264], F32R)
                for pc in _PIECES:
                    t = strip[pc["tid"]]
                    ph, cbase = sdim[pc["tid"]]
                    off = ((pc["c0"] - cbase) * ph * PW
                           + (R + pc["di"]) * PW + pc["dj"])
                    src = bass.AP(
                        t,
                        off,
                        [[1, pc["ntap"]], [ph * PW, pc["nch"]], [1, 520]],
                    )
                    npart = pc["ntap"] * pc["nch"]
                    # dst: partitions p..p+npart, contiguous 520-elem span
                    # starting at [g, 0, 0]
                    dst = feat[pc["p"]:pc["p"] + npart, pc["g"], :, :]
                    dst = bass.AP(dst.tensor, dst.offset,
                                  [list(dst.ap[0]), [1, 520]])
                    nc.sync.dma_start(dst, src)
                # bias feature row: constant 1.0 (W1 row 1408 = b1)
                brow = feat[0:1, NKG - 1, :, :]
                brow = bass.AP(brow.tensor, brow.offset,
                               [list(brow.ap[0]), [1, 520]])
                nc.scalar.dma_start(brow, ones.ap()[:])

                ps2 = ps2pool.tile([6, 512], F32)
                for m in range(NM):
                    if pb == 0:
                        load_w1_chunk(m)
                    ps = ps1pool.tile([128, 512], F32)
                    for g in range(NKG):
                        kg = 128 if g < NKG - 1 else KG_LAST
                        nc.tensor.matmul(
                            ps[:],
                            w1_sb[0:kg, g, m * 128:(m + 1) * 128],
                            feat[0:kg, g, :, 0:256],
                            start=(g == 0),
                            stop=(g == NKG - 1),
                        )
                    h = hpool.tile([128, 512], F32R)
                    nc.scalar.activation(
                        h[:], ps[:], mybir.ActivationFunctionType.Relu,
                        bias=zb[:],
                    )
                    nc.tensor.matmul(
                        ps2[:],
                        w2_sb[:, m, 0:6],
                        h[:],
                        start=(m == 0),
                        stop=(m == NM - 1),
                        skip_group_check=True,
                    )

                # All transform ops run on partitions 0:6 with per-partition
                # scale/bias vectors; rows that don't apply get neutral values
                # (scale 0, bias 1) so every lane stays finite.
                # mu rows 0:3: (raw + b2_mu)*255 ; d rows 3:6: 1.1-(raw+b2_s)
                outm = spool.tile([6, 512], F16, tag="outm")
                nc.scalar.activation(
                    outm[:], ps2[:],
                    mybir.ActivationFunctionType.Identity,
                    bias=b3_sb[:, 1:2], scale=b3_sb[:, 0:1],
                )
                d = spool.tile([6, 512], F32, tag="d")
                nc.scalar.activation(
                    d[:], ps2[:],
                    mybir.ActivationFunctionType.Identity,
                    bias=b3_sb[:, 3:4], scale=b3_sb[:, 2:3],
                )
                r = spool.tile([6, 512], F32, tag="r")
                nc.vector.reciprocal(r[:], d[:])
                # n rows 3:6: 100*(raw + b2_s)
                n = spool.tile([6, 512], F32, tag="n")
                nc.scalar.activation(
                    n[:], ps2[:],
                    mybir.ActivationFunctionType.Identity,
                    bias=b3_sb[:, 5:6], scale=b3_sb[:, 4:5],
                )
                sc = spool.tile([6, 512], F32, tag="sc")
                nc.vector.tensor_mul(sc[:], n[:], r[:])
                scc = spool.tile([6, 512], F16, tag="scc")
                nc.vector.tensor_scalar(
                    scc[:], sc[:], 1000.0, 1e-8,
                    op0=mybir.AluOpType.min, op1=mybir.AluOpType.max,
                )
                pbs = slice(pb * 512, (pb + 1) * 512)
                nc.scalar.dma_start(o.ap()[0:3, pbs], outm[0:3, :])
                nc.scalar.dma_start(o.ap()[3:6, pbs], scc[3:6, :])

    nc.compile()
    return nc


_NC_CACHE = None


def _get_nc():
    global _NC_CACHE
    if _NC_CACHE is None:
        _NC_CACHE = _build_nc()
    return _NC_CACHE


def _prep_host_inputs(x, x_ups, W1, b1, W2, b2):
    x = np.asarray(x)
    x_ups = np.asarray(x_ups)
    # halo-padded full images
    def pad_full(a, c0=0):
        nch = C - c0
        p = np.zeros((nch, H + 2 * PAD, PW), np.float32)
        p[:, PAD:PAD + H, PAD:PAD + W] = a[0, c0:]
        return p

    xp = pad_full(x)
    up = pad_full(x_ups, CU0)

    # reordered W1; bias row (=b1) at position 1408; zero-pad to 12*128 rows
    W1g = np.zeros((W1ROWS, HID), np.float32)
    W1p = np.asarray(W1)[_PERM]
    W1g[:1408] = W1p[:1408]
    W1g[1408] = np.asarray(b1)
    W1g[1409:1471] = W1p[1408:]
    # W2 column-reordered: [mu0 mu1 mu2 s0 s1 s2]
    W2a = np.asarray(W2).astype(np.float32)
    b2a = np.asarray(b2).astype(np.float32)
    W2r = np.ascontiguousarray(W2a[:, [0, 2, 4, 1, 3, 5]])
    b3 = np.zeros((6, 6), np.float32)
    b3[0:3, 0] = 255.0                        # sA (mu scale)
    b3[0:3, 1] = b2a[[0, 2, 4]] * 255.0       # bA (mu bias)
    b3[3:6, 2] = -1.0                         # sC (d scale)
    b3[0:3, 3] = 1.0                          # bC neutral rows
    b3[3:6, 3] = 1.1 - b2a[[1, 3, 5]]         # bC (d bias)
    b3[3:6, 4] = 100.0                        # sD (n scale)
    b3[0:3, 5] = 1.0                          # bD neutral rows
    b3[3:6, 5] = 100.0 * b2a[[1, 3, 5]]       # bD (n bias)

    in_maps = []
    for k in range(NCORES):
        r0 = k * ROWS_PER_CORE
        in_maps.append({
            "xs": np.ascontiguousarray(xp[:, r0:r0 + PH_X, :]),
            "us": np.ascontiguousarray(up[:, r0:r0 + PH_U, :]),
            "w1s": np.ascontiguousarray(W1g[k * W1SH:(k + 1) * W1SH]),
            "w2": W2r,
            "b3": b3,
            "ones": np.ones((1, 520), np.float32),
        })
    return in_maps


def kernel(x, x_ups, W1, b1, W2, b2):
    nc = _get_nc()
    in_maps = _prep_host_inputs(x, x_ups, W1, b1, W2, b2)
    res = run_bass_kernel_spmd(nc, in_maps, core_ids=list(range(NCORES)))
    ocs = np.stack([res.results[k]["o"] for k in range(NCORES)])  # (8, 6, 8192)
    flat = ocs.transpose(0, 2, 1).reshape(H * W, 6)               # (65536, 6)
    out = flat.reshape(H * W, 2, 3).transpose(0, 2, 1)            # (65536, 3, 2)
    return np.ascontiguousarray(out[None]).astype(np.float32)     # (1, 65536, 3, 2)


# revision 13
# speedup vs baseline: 4.2213x; 1.0063x over previous
"""Trainium2 Bass kernel for nn_ARM_28217935134778 (dense_cnn).

Computation (see reference): for each of the 65536 pixels of a 256x256 image,
gather a 7x7 window over 30 channels from two tensors (x: first 24 taps per
channel, x_ups: flat-tail 750 taps), feed the 1470-dim feature through a
1470 -> 2048 -> 6 MLP (ReLU in the middle), then map the 6 outputs to
(mu, scale) pairs.

Implementation: implicit-GEMM convolution, data-parallel over 8 NeuronCores
(each core takes a 32-row horizontal strip of the image, with the needed halo
rows baked into its padded input strip on the host).

The dominant cost under the axon tunnel is host->device staging (~37 MB/s), so
inputs are kept minimal:
  - W1 is shipped SHARDED (192 rows per core) and AllGathered on-device over
    NeuronLink, instead of replicating the 12.6 MB reordered matrix 8x.
  - The x strip only carries the 35 rows that causal taps 0..23 can touch;
    the x_ups strip only carries channels 14..29 (the flat tail [:, 720:]
    never reads channels 0..13).

Device layout per core:
  - rhs "feature" tiles [128 feats, 512 pixels] built by shifted DMAs from the
    halo-padded strips (one DMA covers several taps x channels).
  - W1 is host-reordered so its rows match the feature order; b1 is folded
    in via an extra constant-1.0 feature row whose W1 row equals b1.
  - Layer 1: out1[hid, pix] accumulated over 12 K-groups per 128-hid block
    (float32r matmuls, full PE rate). ReLU evict PSUM->SBUF on ACT.
  - Layer 2: out2[6, pix] accumulated over the 16 hidden chunks.
  - mu/scale transform on ACT/DVE, output stored feature-major (6, 8192);
    the host transposes/interleaves while gathering.
"""

import numpy as np

try:
    # The repeat-call cost under axon is dominated by staging + the fresh
    # jax.jit that run_bass_kernel_spmd builds per call; the persistent
    # compilation cache turns the per-call XLA re-compile into a disk hit.
    import tempfile as _tempfile
    import jax as _jax
    _jax.config.update("jax_compilation_cache_dir",
                       _tempfile.gettempdir() + "/jax_comp_cache")
    _jax.config.update("jax_persistent_cache_min_entry_size_bytes", -1)
    _jax.config.update("jax_persistent_cache_min_compile_time_secs", 0.0)
except Exception:
    pass

import concourse.bass as bass
import concourse.mybir as mybir
import concourse.tile as tile
from concourse import bacc
from concourse.bass_utils import run_bass_kernel_spmd

F32 = mybir.dt.float32
F32R = mybir.dt.float32r
F16 = mybir.dt.float16

C = 30            # channels
H = W = 256
KW = 7            # window
PAD = 3
CENTER = 24       # causal taps per channel
HID = 2048
NCORES = 8
ROWS_PER_CORE = H // NCORES          # 32
PIX_PER_CORE = ROWS_PER_CORE * W     # 8192
PW = 264                             # padded row width (3 left, 5 right)
PH_X = ROWS_PER_CORE + PAD           # 35 rows: x taps only reach di 0..3
PH_U = ROWS_PER_CORE + 2 * PAD       # 38 rows for the full-window ups taps
CU0 = 14                             # first x_ups channel actually used
CU = C - CU0                         # 16 shipped x_ups channels
NPB = PIX_PER_CORE // 512            # 16 pixel blocks (2 image rows each)
NKG = 12                             # K groups (11 x 128 + 1 x 63)
KG_LAST = 63                         # 62 feature rows + 1 bias row
NM = HID // 128                      # 16 hidden blocks
W1SH = 184                           # W1 shard rows shipped per core
W1ROWS = NCORES * W1SH               # 1472 gathered rows (1471 used)
W2SH = HID * 6 // NCORES             # 1536-float W2 shard per core


def _build_runs():
    """Feature rows in our contraction order: (tensor_id, di, dj, c0, nch)."""
    runs = []
    for t in range(CENTER):                       # x: taps 0..23, all 30 ch
        runs.append((0, t // KW, t % KW, 0, C))
    for t in range(KW * KW):                      # x_ups tail
        c0 = 15 if t < 34 else 14
        runs.append((1, t // KW, t % KW, c0, C - c0))
    return runs


def _build_perm(runs):
    """Original W1 row index for each position in our feature order."""
    perm = []
    for (tid, di, dj, c0, nch) in runs:
        t = di * KW + dj
        for c in range(c0, c0 + nch):
            perm.append(c * CENTER + t if tid == 0 else c * KW * KW + t)
    assert len(perm) == 1470
    assert sorted(perm) == list(range(1470))
    return perm


def _build_pieces(runs):
    """Split runs at 128-row group boundaries, then merge consecutive taps
    (same di, channel range) into single multi-tap DMA pieces.

    Position 1408 (partition 0 of K-group 11) is reserved for the constant-1
    bias feature row, so feature positions >= 1408 shift up by one."""
    subs = []
    pos = 0
    for (tid, di, dj, c0, nch) in runs:
        left, cs = nch, c0
        while left:
            g, p = divmod(pos if pos < 1408 else pos + 1, 128)
            take = min(left, 128 - p)
            subs.append(dict(g=g, p=p, tid=tid, di=di, dj=dj, c0=cs, nch=take))
            pos += take
            cs += take
            left -= take
    assert pos == 1470
    pieces = []
    for s in subs:
        m = pieces[-1] if pieces else None
        if (m is not None and m["g"] == s["g"] and m["tid"] == s["tid"]
                and m["di"] == s["di"] and m["c0"] == s["c0"]
                and m["nch"] == s["nch"] and s["dj"] == m["dj"] + m["ntap"]
                and s["p"] == m["p"] + m["ntap"] * m["nch"]):
            m["ntap"] += 1
        else:
            pieces.append(dict(**s, ntap=1))
    return pieces


_RUNS = _build_runs()
_PERM = _build_perm(_RUNS)
_PIECES = _build_pieces(_RUNS)


def _build_nc(fbufs=2, hbufs=4, ps1bufs=7, ps2bufs=1, npb=NPB):
    nc = bacc.Bacc("TRN2", target_bir_lowering=False, debug=False,
                   num_devices=NCORES)
    xs = nc.dram_tensor("xs", (C, PH_X, PW), F32R, kind="ExternalInput")
    us = nc.dram_tensor("us", (CU, PH_U, PW), F32R, kind="ExternalInput")
    # this core's 184-row shard of the reordered W1; AllGathered on-device
    w1s = nc.dram_tensor("w1s", (W1SH, HID), F32R, kind="ExternalInput")
    # this core's 1536-float slice of the flat reordered W2 (2048 x 6,
    # columns [mu0 mu1 mu2 s0 s1 s2]); AllGathered on-device
    w2s = nc.dram_tensor("w2s", (W2SH,), F32R, kind="ExternalInput")
    # per-partition scale/bias vectors for the output transform
    # cols: sA bA (mu), sC bC (denominator), sD bD (numerator)
    b3 = nc.dram_tensor("b3", (6, 6), F32, kind="ExternalInput")
    ones = nc.dram_tensor("ones", (1, 520), F32R, kind="ExternalInput")
    # fp16 output: halves the donated-zero upload and the result fetch through
    # the tunnel; adds at most 0.25 absolute rounding on values <= 1000.
    o = nc.dram_tensor("o", (6, PIX_PER_CORE), F16, kind="ExternalOutput")
    strip = {0: xs, 1: us}
    sdim = {0: (PH_X, 0), 1: (PH_U, CU0)}

    with tile.TileContext(nc) as tc:
        with (
            tc.tile_pool(name="dpool", bufs=1, space="DRAM") as dpool,
            tc.tile_pool(name="wpool", bufs=1) as wpool,
            tc.tile_pool(name="cpool", bufs=1) as cpool,
            tc.tile_pool(name="fpool", bufs=fbufs) as fpool,
            tc.tile_pool(name="hpool", bufs=hbufs) as hpool,
            tc.tile_pool(name="spool", bufs=1) as spool,
            tc.tile_pool(name="opool", bufs=2) as opool,
            tc.tile_pool(name="ps1pool", bufs=ps1bufs, space="PSUM") as ps1pool,
            tc.tile_pool(name="ps2pool", bufs=ps2bufs, space="PSUM") as ps2pool,
        ):
            # --- W1 AllGather: shard (184, 2048) per core -> full (1472, 2048)
            w1_cc_in = dpool.tile([W1SH, HID], F32R)
            nc.sync.dma_start(w1_cc_in[:], w1s.ap()[:])
            w1_full = dpool.tile([W1ROWS, HID], F32R, addr_space="Shared")
            nc.gpsimd.collective_compute(
                "AllGather",
                mybir.AluOpType.bypass,
                replica_groups=[list(range(NCORES))],
                ins=[w1_cc_in[:].opt()],
                outs=[w1_full[:].opt()],
            )
            # --- W2 AllGather: 1536 floats per core -> flat (2048 x 6)
            w2_cc_in = dpool.tile([W2SH], F32R)
            nc.sync.dma_start(w2_cc_in[:], w2s.ap()[:])
            w2_full = dpool.tile([HID * 6], F32R, addr_space="Shared")
            nc.gpsimd.collective_compute(
                "AllGather",
                mybir.AluOpType.bypass,
                replica_groups=[list(range(NCORES))],
                ins=[w2_cc_in[:].opt()],
                outs=[w2_full[:].opt()],
            )

            w1_sb = wpool.tile([128, NKG, HID], F32R)

            # m-major lazy load: chunk m (all K-groups, one 128-wide hidden
            # block, 736KB) is issued just before pixel-block 0 consumes it,
            # so the matmul stream pipelines against the AllGather instead of
            # waiting for the whole 12MB. Gathered row g*128+p; the last
            # K-group only has 64 valid rows (1408..1471), so it loads as a
            # separate piece.
            def load_w1_chunk(m):
                base = m * 128
                nc.sync.dma_start(
                    w1_sb[:, 0:11, m * 128:(m + 1) * 128],
                    bass.AP(w1_full.tensor, w1_full.offset + base,
                            [[HID, 128], [128 * HID, 11], [1, 128]]))
                nc.sync.dma_start(
                    w1_sb[0:64, 11, m * 128:(m + 1) * 128],
                    bass.AP(w1_full.tensor,
                            w1_full.offset + 11 * 128 * HID + base,
                            [[HID, 64], [1, 128]]))
            # chunk stride padded to 8 floats so each lhsT slice is 32B-aligned
            w2_sb = wpool.tile([128, NM, 8], F32R)
            nc.sync.dma_start(w2_sb[:, :, 0:6],
                              bass.AP(w2_full.tensor, w2_full.offset,
                                      [[6, 128], [768, NM], [1, 6]]))
            b3_sb = cpool.tile([6, 6], F32)
            nc.sync.dma_start(b3_sb[:], b3.ap()[:])
            zb = cpool.tile([128, 1], F32)
            nc.any.memset(zb[:], 0.0)

            for pb in range(npb):
                R = 2 * pb  # first image row (strip-local) of this block
                # free layout per K-row: [2 rows, 264 cols] — a single
                # contiguous 520-element DRAM read covers both rows (the 8
                # inter-row pad columns land in [*, 0, 256:264] and are never
                # read by the matmuls).
                feat = fpool.tile([128, NKG, 2, 264], F32R)
                for pc in _PIECES:
                    t = strip[pc["tid"]]
                    ph, cbase = sdim[pc["tid"]]
                    off = ((pc["c0"] - cbase) * ph * PW
                           + (R + pc["di"]) * PW + pc["dj"])
                    src = bass.AP(
                        t,
                        off,
                        [[1, pc["ntap"]], [ph * PW, pc["nch"]], [1, 520]],
                    )
                    npart = pc["ntap"] * pc["nch"]
                    # dst: partitions p..p+npart, contiguous 520-elem span
                    # starting at [g, 0, 0]
                    dst = feat[pc["p"]:pc["p"] + npart, pc["g"], :, :]
                    dst = bass.AP(dst.tensor, dst.offset,
                                  [list(dst.ap[0]), [1, 520]])
                    nc.sync.dma_start(dst, src)
                # bias feature row: constant 1.0 (W1 row 1408 = b1)
                brow = feat[0:1, NKG - 1, :, :]
                brow = bass.AP(brow.tensor, brow.offset,
                               [list(brow.ap[0]), [1, 520]])
                nc.scalar.dma_start(brow, ones.ap()[:])

                ps2 = ps2pool.tile([6, 512], F32)
                for m in range(NM):
                    if pb == 0:
                        load_w1_chunk(m)
                    ps = ps1pool.tile([128, 512], F32)
                    for g in range(NKG):
                        kg = 128 if g < NKG - 1 else KG_LAST
                        nc.tensor.matmul(
                            ps[:],
                            w1_sb[0:kg, g, m * 128:(m + 1) * 128],
                            feat[0:kg, g, :, 0:256],
                            start=(g == 0),
                            stop=(g == NKG - 1),
                        )
                    h = hpool.tile([128, 512], F32R)
                    nc.scalar.activation(
                        h[:], ps[:], mybir.ActivationFunctionType.Relu,
                        bias=zb[:],
                    )
                    nc.tensor.matmul(
                        ps2[:],
                        w2_sb[:, m, 0:6],
                        h[:],
                        start=(m == 0),
                        stop=(m == NM - 1),
                        skip_group_check=True,
                    )

                # All transform ops run on partitions 0:6 with per-partition
                # scale/bias vectors; rows that don't apply get neutral values
                # (scale 0, bias 1) so every lane stays finite.
                # mu rows 0:3: (raw + b2_mu)*255 ; d rows 3:6: 1.1-(raw+b2_s)
                outm = spool.tile([6, 512], F16, tag="outm")
                nc.scalar.activation(
                    outm[:], ps2[:],
                    mybir.ActivationFunctionType.Identity,
                    bias=b3_sb[:, 1:2], scale=b3_sb[:, 0:1],
                )
                d = spool.tile([6, 512], F32, tag="d")
                nc.scalar.activation(
                    d[:], ps2[:],
                    mybir.ActivationFunctionType.Identity,
                    bias=b3_sb[:, 3:4], scale=b3_sb[:, 2:3],
                )
                r = spool.tile([6, 512], F32, tag="r")
                nc.vector.reciprocal(r[:], d[:])
                # n rows 3:6: 100*(raw + b2_s)
                n = spool.tile([6, 512], F32, tag="n")
                nc.scalar.activation(
                    n[:], ps2[:],
                    mybir.ActivationFunctionType.Identity,
                    bias=b3_sb[:, 5:6], scale=b3_sb[:, 4:5],
                )
                sc = spool.tile([6, 512], F32, tag="sc")
                nc.vector.tensor_mul(sc[:], n[:], r[:])
                scc = spool.tile([6, 512], F16, tag="scc")
                nc.vector.tensor_scalar(
                    scc[:], sc[:], 1000.0, 1e-8,
                    op0=mybir.AluOpType.min, op1=mybir.AluOpType.max,
                )
                pbs = slice(pb * 512, (pb + 1) * 512)
                nc.scalar.dma_start(o.ap()[0:3, pbs], outm[0:3, :])
                nc.scalar.dma_start(o.ap()[3:6, pbs], scc[3:6, :])

    nc.compile()
    return nc


_NC_CACHE = None


def _get_nc():
    global _NC_CACHE
    if _NC_CACHE is None:
        _NC_CACHE = _build_nc()
    return _NC_CACHE


def _prep_host_inputs(x, x_ups, W1, b1, W2, b2):
    x = np.asarray(x)
    x_ups = np.asarray(x_ups)
    # halo-padded full images
    def pad_full(a, c0=0):
        nch = C - c0
        p = np.zeros((nch, H + 2 * PAD, PW), np.float32)
        p[:, PAD:PAD + H, PAD:PAD + W] = a[0, c0:]
        return p

    xp = pad_full(x)
    up = pad_full(x_ups, CU0)

    # reordered W1; bias row (=b1) at position 1408; zero-pad to 1472 rows
    W1g = np.zeros((W1ROWS, HID), np.float32)
    W1p = np.asarray(W1)[_PERM]
    W1g[:1408] = W1p[:1408]
    W1g[1408] = np.asarray(b1)
    W1g[1409:1471] = W1p[1408:]
    # W2 column-reordered: [mu0 mu1 mu2 s0 s1 s2]
    W2a = np.asarray(W2).astype(np.float32)
    b2a = np.asarray(b2).astype(np.float32)
    W2r = np.ascontiguousarray(W2a[:, [0, 2, 4, 1, 3, 5]])
    W2f = W2r.ravel()
    b3 = np.zeros((6, 6), np.float32)
    b3[0:3, 0] = 255.0                        # sA (mu scale)
    b3[0:3, 1] = b2a[[0, 2, 4]] * 255.0       # bA (mu bias)
    b3[3:6, 2] = -1.0                         # sC (d scale)
    b3[0:3, 3] = 1.0                          # bC neutral rows
    b3[3:6, 3] = 1.1 - b2a[[1, 3, 5]]         # bC (d bias)
    b3[3:6, 4] = 100.0                        # sD (n scale)
    b3[0:3, 5] = 1.0                          # bD neutral rows
    b3[3:6, 5] = 100.0 * b2a[[1, 3, 5]]       # bD (n bias)

    in_maps = []
    for k in range(NCORES):
        r0 = k * ROWS_PER_CORE
        in_maps.append({
            "xs": np.ascontiguousarray(xp[:, r0:r0 + PH_X, :]),
            "us": np.ascontiguousarray(up[:, r0:r0 + PH_U, :]),
            "w1s": np.ascontiguousarray(W1g[k * W1SH:(k + 1) * W1SH]),
            "w2s": np.ascontiguousarray(W2f[k * W2SH:(k + 1) * W2SH]),
            "b3": b3,
            "ones": np.ones((1, 520), np.float32),
        })
    return in_maps


def kernel(x, x_ups, W1, b1, W2, b2):
    nc = _get_nc()
    in_maps = _prep_host_inputs(x, x_ups, W1, b1, W2, b2)
    res = run_bass_kernel_spmd(nc, in_maps, core_ids=list(range(NCORES)))
    ocs = np.stack([res.results[k]["o"] for k in range(NCORES)])  # (8, 6, 8192)
    flat = ocs.transpose(0, 2, 1).reshape(H * W, 6)               # (65536, 6)
    out = flat.reshape(H * W, 2, 3).transpose(0, 2, 1)            # (65536, 3, 2)
    return np.ascontiguousarray(out[None]).astype(np.float32)     # (1, 65536, 3, 2)
